# revision 13
# baseline (speedup 1.0000x reference)
"""Trainium2 Bass kernel for a dense transformer block (B=128, T=256, C=384, H=6).

Sharding: data-parallel over batch across 8 NeuronCores (16 batches/core),
identical SPMD program per core, no collectives.

Design (v3):
  - per-core schedule: batches in pairs (free dim 512 in the big matmuls),
    software-pipelined: pair p's attention exp chain (ACT/DVE) drains behind
    pair p-1's MLP (PE); input DMAs prefetched two pairs ahead so they are
    not stuck behind output DMAs in the sync queue.
  - exp on the scalar (ACT) engine (exact); causal mask + rowsum as one
    all-bf16 DVE scalar_tensor_tensor (2x rate) with accum_out.
  - 1/rowsum folded into the attention-weight transpose: regular all-bf16
    matmul with diag(1/rowsum) as the moving operand (1 cyc/row).
  - all transposes are regular matmuls against a bf16 identity (1 cyc/row).
  - relu and LN-apply on ACT (Relu / Identity with per-row scale+bias);
    LN stats + rstd (ACT Sqrt + DVE reciprocal) on DVE.
  - everything bf16 except the residual stream (x, y, out f32) and PSUM.
  - scores matmuls (K=64) must go to separate PSUM banks: two back-to-back
    K=64 matmuls into one bank crash the device (sub-array drain collision).
  - MLP down-projection i-outer so PSUM needs 1 rotating bank, not 4 held.
"""

import numpy as np

import concourse.bass as bass
import concourse.mybir as mybir
from concourse import bacc
from concourse.tile import TileContext
from contextlib import ExitStack

B, T, C = 128, 256, 384
H, D = 6, 64
FF = 4 * C
NCORES = 8
BL = B // NCORES  # 16
NPAIR = BL // 2  # 8
KC = C // 128  # 3
KH = FF // 128  # 12
EPS = 1e-5
F32 = mybir.dt.float32
F32R = mybir.dt.float32r
BF16 = mybir.dt.bfloat16
I32 = mybir.dt.int32
F8 = mybir.dt.float8e4
ALU = mybir.AluOpType
ACTF = mybir.ActivationFunctionType

EXP_S = float(2**23 / np.log(2.0))
EXP_B = float(127 * 2**23)
MASKB = 4.0e8  # masked scores -> it ~ 4e8 -> bitcast float ~1e-21 (safe to |s|<33)
SQRT_MAGIC = 0x1FBD1DF5
_STAGE = 99  # debug: truncate program after stage N (99 = full)


def build_program(use_g1, use_b1ln, use_g2, use_b2ln, use_bp, use_b1, use_b2):
    nc = bacc.Bacc(None)
    x = nc.declare_dram_parameter("x", [BL, T, C], F32, isOutput=False)
    # packed weights: [C, wq|wk|wv|wp] so one DMA trigger loads all four
    wqkvp = nc.declare_dram_parameter("wqkvp", [C, 4 * C], BF16, isOutput=False)
    w1 = nc.declare_dram_parameter("w1", [C, FF], BF16, isOutput=False)
    w2 = nc.declare_dram_parameter("w2", [FF, C], BF16, isOutput=False)
    g1 = nc.declare_dram_parameter("g1", [128, C], F32, isOutput=False)
    b1ln = nc.declare_dram_parameter("b1ln", [128, C], F32, isOutput=False)
    g2 = nc.declare_dram_parameter("g2", [128, C], F32, isOutput=False)
    b2ln = nc.declare_dram_parameter("b2ln", [128, C], F32, isOutput=False)
    bpb = nc.declare_dram_parameter("bpb", [128, C], F32, isOutput=False)
    b2b = nc.declare_dram_parameter("b2b", [128, C], F32, isOutput=False)
    b1c = nc.declare_dram_parameter("b1c", [128, KH], F32, isOutput=False)
    # packed constants: mask0 [0:256] | mask1 [256:768] | identb [768:896]
    consts = nc.declare_dram_parameter("consts", [128, 896], BF16, isOutput=False)
    out = nc.declare_dram_parameter("out", [BL, T, C], F32, isOutput=True)

    with TileContext(nc) as tc, ExitStack() as ctx:
        wts = ctx.enter_context(tc.tile_pool(name="wts", bufs=1))
        sb = ctx.enter_context(tc.tile_pool(name="sb", bufs=1))
        st = ctx.enter_context(tc.tile_pool(name="st", bufs=4))
        tr = ctx.enter_context(tc.tile_pool(name="tr", bufs=4))
        ps = ctx.enter_context(tc.tile_pool(name="ps", bufs=7, space="PSUM"))
        psy = ctx.enter_context(tc.tile_pool(name="psy", bufs=1, space="PSUM"))

        def load_one(dram, shape, tag, dt=F32):
            t_ = wts.tile(shape, dt, name=tag, tag=tag)
            nc.sync.dma_start(out=t_, in_=dram[:, :])
            return t_

        # ---- batched input DMA plan (one trigger each; sync-engine issue
        # cost is ~600ns/trigger so fewer, bigger triggers win) ----
        consts_sb = load_one(consts, [128, 896], "consts", dt=BF16)
        mask0_sb = consts_sb[:, 0:256]
        mask1_sb = consts_sb[:, 256:768]
        id_bf = consts_sb[:, 768:896]

        xp_tiles = [None] * NPAIR

        def prefetch(p):
            xp = sb.tile([128, 4 * C], F32, name="xp", tag="xp", bufs=4)
            nc.sync.dma_start(
                out=xp.rearrange("p (b t c) -> p b t c", b=2, t=2),
                in_=x[2 * p : 2 * p + 2, :, :].rearrange(
                    "b (t q) c -> q b t c", q=128
                ),
            )
            xp_tiles[p] = xp
            return [xp[:, i * C : (i + 1) * C] for i in range(4)]

        xts = [None] * NPAIR
        xts[0] = prefetch(0)
        xts[1] = prefetch(1)

        wqkvp_sb = wts.tile([128, KC * 4 * C], BF16, name="wqkvp", tag="wqkvp")
        nc.sync.dma_start(
            out=wqkvp_sb.rearrange("p (k f) -> p k f", k=KC),
            in_=wqkvp.rearrange("(k p) f -> p k f", p=128),
        )
        w3 = wqkvp_sb.rearrange("p (k f) -> p k f", k=KC)
        wq_sb = [w3[:, k, 0:C] for k in range(KC)]
        wk_sb = [w3[:, k, C : 2 * C] for k in range(KC)]
        wv_sb = [w3[:, k, 2 * C : 3 * C] for k in range(KC)]
        wp_sb = [w3[:, k, 3 * C : 4 * C] for k in range(KC)]

        xts[2] = prefetch(2)

        w1t = wts.tile([128, KC * FF], BF16, name="w1t", tag="w1t")
        nc.sync.dma_start(
            out=w1t.rearrange("p (k f) -> p k f", k=KC),
            in_=w1.rearrange("(k p) f -> p k f", p=128),
        )
        w1_3 = w1t.rearrange("p (k f) -> p k f", k=KC)
        w1_sb = [w1_3[:, k, :] for k in range(KC)]

        xts[3] = prefetch(3)

        w2t = wts.tile([128, KH * C], BF16, name="w2t", tag="w2t")
        nc.sync.dma_start(
            out=w2t.rearrange("p (m f) -> p m f", m=KH),
            in_=w2.rearrange("(m p) f -> p m f", p=128),
        )
        w2_3 = w2t.rearrange("p (m f) -> p m f", m=KH)
        w2_sb = [w2_3[:, m, :] for m in range(KH)]

        g1_sb = load_one(g1, [128, C], "g1") if use_g1 else None
        b1ln_sb = load_one(b1ln, [128, C], "b1ln") if use_b1ln else None
        g2_sb = load_one(g2, [128, C], "g2") if use_g2 else None
        b2ln_sb = load_one(b2ln, [128, C], "b2ln") if use_b2ln else None
        bpb_sb = load_one(bpb, [128, C], "bpb") if use_bp else None
        b2b_sb = load_one(b2b, [128, C], "b2b") if use_b2 else None
        b1c_sb = load_one(b1c, [128, KH], "b1c") if use_b1 else None

        def batched_rstd(mv8):
            """[128,8] interleaved (mean,var) x4 -> rstd4 [128,4]."""
            mv_v = mv8.rearrange("p (i two) -> p i two", two=2)
            var4 = mv_v[:, :, 1]
            vpe = st.tile([128, 4], F32, name="vpe", tag="vpe")
            nc.vector.tensor_scalar(
                out=vpe, in0=var4, scalar1=EPS, scalar2=None, op0=ALU.add)
            sd4 = st.tile([128, 4], F32, name="sd4", tag="sd4")
            nc.scalar.activation(sd4, vpe, ACTF.Sqrt)
            rstd4 = st.tile([128, 4], F32, name="rstd4", tag="rstd4")
            nc.vector.reciprocal(rstd4, sd4)
            return rstd4

        def ln_stat(mv8, i, src):
            stats = st.tile([128, 6], F32, name="lst", tag="lst")
            nc.vector.bn_stats(stats, src)
            nc.vector.bn_aggr(mv8[:, 2 * i : 2 * i + 2], stats)

        def layernorm4(dsts, srcs, g_sb, b_sb, mv8=None):
            if mv8 is None:
                mv8 = st.tile([128, 8], F32, name="mv8", tag="mv8")
                for i in range(4):
                    ln_stat(mv8, i, srcs[i])
            rstd4 = batched_rstd(mv8)
            mv_v2 = mv8.rearrange("p (i two) -> p i two", two=2)
            nmr4 = st.tile([128, 4], F32, name="nmr4", tag="nmr4")
            nc.vector.scalar_tensor_tensor(
                out=nmr4, in0=mv_v2[:, :, 0], scalar=-1.0, in1=rstd4,
                op0=ALU.mult, op1=ALU.mult,
            )
            for i in range(4):
                nc.scalar.activation(
                    dsts[i], srcs[i], ACTF.Identity,
                    bias=nmr4[:, i : i + 1], scale=rstd4[:, i : i + 1],
                )
                if g_sb is not None:
                    nc.vector.tensor_mul(dsts[i], dsts[i], g_sb)
                if b_sb is not None:
                    nc.vector.tensor_add(dsts[i], dsts[i], b_sb)

        def transpose4_into(dstT, srcs):
            """4x [128,C] token-major -> dstT [128, KC*2T] C-major packed."""
            dst3 = dstT.rearrange("q (c w) -> q c w", c=KC)
            for i in range(4):
                pt = ps.tile([128, C], F32, name="pa", tag="pa")
                for c in range(KC):
                    nc.tensor.matmul(
                        pt[:, c * 128 : (c + 1) * 128],
                        srcs[i][:, c * 128 : (c + 1) * 128],
                        id_bf,
                        start=True, stop=True,
                    )
                nc.scalar.copy(
                    dst3[:, :, i * 128 : (i + 1) * 128],
                    pt.rearrange("q (c w) -> q c w", c=KC),
                )


        def phase1a(p, xt):
            bs = [2 * p, 2 * p, 2 * p + 1, 2 * p + 1]
            tch = [0, 1, 0, 1]
            hT = sb.tile(
                [128, KC * 2 * T], BF16, name="hT", tag="hT", bufs=3
            )
            ht_ = [
                sb.tile([128, C], BF16, name=f"h{i}", tag=f"h{i}")
                for i in range(4)
            ]
            layernorm4(ht_, xt, g1_sb, b1ln_sb)
            transpose4_into(hT, ht_)

            # ---- stage 2: q^T (f32r), k^T (bf16) C-major; v token-major ----
            qT = [
                sb.tile([128, 2 * T], BF16, name=f"qT{m}", tag=f"qT{m}", bufs=3)
                for m in range(KC)
            ]
            kT = [
                sb.tile([128, 2 * T], BF16, name=f"kT{m}", tag=f"kT{m}", bufs=3)
                for m in range(KC)
            ]
            for m in range(KC):
                pq = ps.tile([128, 2 * T], F32, name="pa", tag="pa")
                for k in range(KC):
                    nc.tensor.matmul(
                        pq, wq_sb[k][:, m * 128 : (m + 1) * 128],
                        hT[:, k * 2 * T : (k + 1) * 2 * T],
                        start=(k == 0), stop=(k == KC - 1),
                    )
                nc.scalar.copy(qT[m], pq)
                pk = ps.tile([128, 2 * T], F32, name="pa", tag="pa")
                for k in range(KC):
                    nc.tensor.matmul(
                        pk, wk_sb[k][:, m * 128 : (m + 1) * 128],
                        hT[:, k * 2 * T : (k + 1) * 2 * T],
                        start=(k == 0), stop=(k == KC - 1),
                    )
                nc.scalar.copy(kT[m], pk)
            vt = [
                sb.tile([128, C], BF16, name=f"v{i}", tag=f"v{i}", bufs=3)
                for i in range(4)
            ]
            for i in range(4):
                pv = ps.tile([128, C], F32, name="pa", tag="pa")
                for k in range(KC):
                    nc.tensor.matmul(
                        pv, hT[:, k * 2 * T + i * 128 : k * 2 * T + (i + 1) * 128],
                        wv_sb[k],
                        start=(k == 0), stop=(k == KC - 1),
                    )
                nc.vector.tensor_copy(vt[i], pv)

            # ---- stage 3: attention ----
            # Two passes: pass A issues all score matmuls + exp chains so the
            # PE streams ahead while DVE/gpsimd chew; pass B does the
            # normalize-transposes and weight application.
            yb0_g, yb1_g, dg_g = [], [], []
            for g in range(2 * KC):
                ib, ch = g // KC, g % KC
                tb = ib * T  # token base of batch ib in 2T-packed tiles
                # NOTE: K=64 matmuls issued back-to-back into the SAME PSUM
                # bank crash the device (concurrent sub-array drains
                # collide); each head gets its own bank.
                # scores tc0: queries 0..127, keys 0..127
                pS0 = [ps.tile([128, 128], F32, name="pa", tag="pa")
                       for _ in range(2)]
                for par in range(2):
                    o = par * 64
                    nc.tensor.matmul(
                        pS0[par],
                        qT[ch][o : o + 64, tb : tb + 128],
                        kT[ch][o : o + 64, tb : tb + 128],
                        start=True, stop=True,
                    )
                ye0 = tr.tile([128, 2 * 128], BF16, name=f"ye0_{g}",
                              tag=f"ye0_{g}", bufs=1)
                for par in range(2):
                    nc.scalar.activation(
                        ye0[:, par * 128 : (par + 1) * 128], pS0[par],
                        ACTF.Exp,
                    )
                # scores tc1: queries 128..255, keys 0..255
                pS1 = [ps.tile([128, T], F32, name="pa", tag="pa")
                       for _ in range(2)]
                for par in range(2):
                    o = par * 64
                    nc.tensor.matmul(
                        pS1[par],
                        qT[ch][o : o + 64, tb + 128 : tb + 2 * 128],
                        kT[ch][o : o + 64, tb : tb + T],
                        start=True, stop=True,
                    )
                ye1 = tr.tile([128, 2 * T], BF16, name=f"ye1_{g}",
                              tag=f"ye1_{g}", bufs=1)
                for par in range(2):
                    nc.scalar.activation(
                        ye1[:, par * T : (par + 1) * T], pS1[par],
                        ACTF.Exp,
                    )
                # causal mask multiply + rowsum, all-bf16 (2x DVE rate)
                rs4 = st.tile([128, 4], F32, name=f"rs4_{g}", tag=f"rs4_{g}", bufs=1)
                yb0 = tr.tile([128, 2 * 128], BF16, name=f"yb0_{g}",
                              tag=f"yb0_{g}", bufs=1)
                yb1 = tr.tile([128, 2 * T], BF16, name=f"yb1_{g}",
                              tag=f"yb1_{g}", bufs=1)
                nc.vector.scalar_tensor_tensor(
                    out=yb0[:, 0:128], in0=ye0[:, 0:128], scalar=1.0,
                    in1=mask0_sb[:, 0:128], op0=ALU.mult, op1=ALU.mult,
                    accum_out=rs4[:, 0:1])
                nc.vector.scalar_tensor_tensor(
                    out=yb0[:, 128:256], in0=ye0[:, 128:256], scalar=1.0,
                    in1=mask0_sb[:, 128:256], op0=ALU.mult, op1=ALU.mult,
                    accum_out=rs4[:, 1:2])
                nc.vector.scalar_tensor_tensor(
                    out=yb1[:, 0:T], in0=ye1[:, 0:T], scalar=1.0,
                    in1=mask1_sb[:, 0:T], op0=ALU.mult, op1=ALU.mult,
                    accum_out=rs4[:, 2:3])
                nc.vector.scalar_tensor_tensor(
                    out=yb1[:, T : 2 * T], in0=ye1[:, T : 2 * T], scalar=1.0,
                    in1=mask1_sb[:, T : 2 * T], op0=ALU.mult, op1=ALU.mult,
                    accum_out=rs4[:, 3:4])
                rr4 = st.tile([128, 4], F32, name=f"rr4_{g}", tag=f"rr4_{g}", bufs=1)
                nc.vector.reciprocal(rr4, rs4)
                # diag(1/rowsum) in bf16 on the (otherwise idle) gpsimd
                dg = [
                    tr.tile([128, 128], BF16, name=f"dg{j}_{g}",
                            tag=f"dg{j}_{g}", bufs=1)
                    for j in range(4)
                ]
                for j in range(4):
                    nc.gpsimd.tensor_scalar(
                        out=dg[j], in0=id_bf, scalar1=rr4[:, j : j + 1],
                        scalar2=None, op0=ALU.mult,
                    )
                yb0_g.append(yb0)
                yb1_g.append(yb1)
                dg_g.append(dg)
            return dict(bs=bs, tch=tch, xt=xt, vt=vt,
                        yb0_g=yb0_g, yb1_g=yb1_g, dg_g=dg_g)

        def phase1b(p, s):
            bs, tch, xt, vt = s["bs"], s["tch"], s["xt"], s["vt"]
            yb0_g, yb1_g, dg_g = s["yb0_g"], s["yb1_g"], s["dg_g"]
            acT = [
                sb.tile([128, 2 * T], BF16, name=f"acT{c}", tag=f"acT{c}",
                        bufs=3)
                for c in range(KC)
            ]
            for g in range(2 * KC):
                ib, ch = g // KC, g % KC
                yb0, yb1, dg = yb0_g[g], yb1_g[g], dg_g[g]
                # transpose attention weights with folded normalization:
                # regular all-bf16 matmul out = yb_slice^T @ diag(rr)
                pA = ps.tile([128, 2 * T], F32, name="pa", tag="pa")
                pB = ps.tile([128, T], F32, name="pa", tag="pa")
                # layout A: [tc0-p0 | tc1-p0-k0 | tc0-p1 | tc1-p1-k0]
                nc.tensor.matmul(pA[:, 0:128], yb0[:, 0:128], dg[0],
                                 start=True, stop=True)
                nc.tensor.matmul(pA[:, 128:256], yb1[:, 0:128], dg[2],
                                 start=True, stop=True)
                nc.tensor.matmul(pA[:, 256:384], yb0[:, 128:256], dg[1],
                                 start=True, stop=True)
                nc.tensor.matmul(pA[:, 384:512], yb1[:, 256:384], dg[3],
                                 start=True, stop=True)
                # layout B: [tc1-p0-k1 | tc1-p1-k1]
                nc.tensor.matmul(pB[:, 0:128], yb1[:, 128:256], dg[2],
                                 start=True, stop=True)
                nc.tensor.matmul(pB[:, 128:256], yb1[:, 384:512], dg[3],
                                 start=True, stop=True)
                wT0 = tr.tile([128, 2 * T], BF16, name="wT0", tag="wT0",
                              bufs=4)
                wT1 = tr.tile([128, T], BF16, name="wT1", tag="wT1",
                              bufs=4)
                nc.scalar.copy(wT0, pA)
                nc.scalar.copy(wT1, pB)
                # apply: pC[par*64:(par+1)*64, q] = sum_k v[k, d] wT[k, q]
                pC = ps.tile([128, T], F32, name="pa", tag="pa")
                for par in range(2):
                    hh = 2 * ch + par
                    o = par * 64
                    nc.tensor.matmul(
                        pC[o : o + 64, 0:T],
                        vt[ib * 2][:, hh * 64 : (hh + 1) * 64],
                        wT0[:, par * T : (par + 1) * T],
                        start=True, stop=False,
                    )
                    nc.tensor.matmul(
                        pC[o : o + 64, 128:T],
                        vt[ib * 2 + 1][:, hh * 64 : (hh + 1) * 64],
                        wT1[:, par * 128 : (par + 1) * 128],
                        start=False, stop=True,
                    )
                nc.vector.tensor_copy(acT[ch][:, ib * T : (ib + 1) * T], pC)

            # ---- stage 4: proj + residual -> y ----
            yt = [
                sb.tile([128, C], F32, name=f"y{i}", tag=f"y{i}", bufs=2)
                for i in range(4)
            ]
            mv8b = st.tile([128, 8], F32, name="mv8", tag="mv8")
            for i in range(4):
                pP = ps.tile([128, C], F32, name="pa", tag="pa")
                for k in range(KC):
                    nc.tensor.matmul(
                        pP, acT[k][:, i * 128 : (i + 1) * 128], wp_sb[k],
                        start=(k == 0), stop=(k == KC - 1),
                    )
                nc.vector.tensor_add(yt[i], pP, xt[i])
                if bpb_sb is not None:
                    nc.vector.tensor_add(yt[i], yt[i], bpb_sb)
                ln_stat(mv8b, i, yt[i])

            h2_ = [
                sb.tile([128, C], BF16, name=f"h2{i}", tag=f"h2{i}", bufs=2)
                for i in range(4)
            ]
            layernorm4(h2_, yt, g2_sb, b2ln_sb, mv8=mv8b)
            return dict(bs=bs, tch=tch, yt=yt, h2_=h2_)

        def phase2(p, s):
            bs, tch, yt, h2_ = s["bs"], s["tch"], s["yt"], s["h2_"]
            h2T = sb.tile(
                [128, KC * 2 * T], BF16, name="h2T", tag="h2T", bufs=2
            )
            transpose4_into(h2T, h2_)
            # ---- stage 6: MLP up + relu ----
            m1r = sb.tile([128, KH * 2 * T], BF16, name="m1r", tag="m1r")
            m1r3 = m1r.rearrange("p (m n) -> p m n", m=KH)
            for m in range(KH):
                pM = ps.tile([128, 2 * T], F32, name="pa", tag="pa")
                for k in range(KC):
                    nc.tensor.matmul(
                        pM, w1_sb[k][:, m * 128 : (m + 1) * 128],
                        h2T[:, k * 2 * T : (k + 1) * 2 * T],
                        start=(k == 0), stop=(k == KC - 1),
                    )
                if m % 2 == 0:
                    nc.scalar.activation(
                        m1r3[:, m, :], pM, ACTF.Relu,
                        bias=(b1c_sb[:, m : m + 1] if use_b1 else 0.0),
                    )
                else:
                    nc.vector.tensor_scalar(
                        out=m1r3[:, m, :], in0=pM,
                        scalar1=(b1c_sb[:, m : m + 1] if use_b1 else 0.0),
                        scalar2=0.0, op0=ALU.add, op1=ALU.max,
                    )

            # ---- stage 7: MLP down (i-outer) + residual + store ----
            otp = sb.tile([128, 4 * C], F32, name="otp", tag="otp", bufs=2)
            for i in range(4):
                # padded to 512 so each psY slot is bank-aligned (mm out
                # must not cross a 2KB PSUM bank)
                pY = psy.tile([128, 512], F32, name="psY", tag="psY")
                for m in range(KH):
                    nc.tensor.matmul(
                        pY[:, 0:C],
                        m1r3[:, m, i * 128 : (i + 1) * 128], w2_sb[m],
                        start=(m == 0), stop=(m == KH - 1),
                    )
                ot = otp[:, i * C : (i + 1) * C]
                nc.vector.tensor_add(ot, pY[:, 0:C], yt[i])
                if b2b_sb is not None:
                    nc.vector.tensor_add(ot, ot, b2b_sb)
            nc.sync.dma_start(
                out=out[2 * p : 2 * p + 2, :, :].rearrange(
                    "b (t q) c -> q b t c", q=128
                ),
                in_=otp.rearrange("p (b t c) -> p b t c", b=2, t=2),
            )

        # Staggered schedule: pair p-1's MLP (pure PE streaming) sits
        # BETWEEN 1a(p) (scores issued) and 1b(p) (needs exp/mask output),
        # so the PE chews MLP while pair p's exp chains drain on ACT/DVE.
        sa = [None] * NPAIR
        sb_ = [None] * NPAIR
        sa[0] = phase1a(0, xts[0])
        sb_[0] = phase1b(0, sa[0])
        for p in range(1, NPAIR):
            if p + 3 < NPAIR:
                xts[p + 3] = prefetch(p + 3)
            sa[p] = phase1a(p, xts[p])
            phase2(p - 1, sb_[p - 1])
            sb_[p] = phase1b(p, sa[p])
        phase2(NPAIR - 1, sb_[NPAIR - 1])

    nc.compile()
    return nc


def _host_prep(inputs):
    f = np.float32
    x = np.ascontiguousarray(inputs["x"], dtype=f)
    import ml_dtypes as _md

    _bf = _md.bfloat16
    wq_full = np.ascontiguousarray(
        (np.asarray(inputs["wq"], dtype=f).transpose(1, 0, 2).reshape(C, C)
         * (C ** -0.5)).astype(_bf)
    )
    wk_full = np.ascontiguousarray(
        np.asarray(inputs["wk"], dtype=f).transpose(1, 0, 2)
        .reshape(C, C).astype(_bf)
    )
    wv_full = np.ascontiguousarray(
        np.asarray(inputs["wv"], dtype=f).transpose(1, 0, 2)
        .reshape(C, C).astype(_bf)
    )
    import ml_dtypes

    bf = ml_dtypes.bfloat16
    wp = np.ascontiguousarray(np.asarray(inputs["w_proj"], dtype=f).astype(bf))
    wqkvp = np.ascontiguousarray(
        np.concatenate([wq_full, wk_full, wv_full, wp], axis=1))
    w1 = np.ascontiguousarray(np.asarray(inputs["w1"], dtype=f).astype(bf))
    w2 = np.ascontiguousarray(np.asarray(inputs["w2"], dtype=f).astype(bf))
    tile128 = lambda v: np.ascontiguousarray(
        np.broadcast_to(np.asarray(v, dtype=f), (128, C))
    )
    g1 = tile128(inputs["ln1_g"])
    b1ln = tile128(inputs["ln1_b"])
    g2 = tile128(inputs["ln2_g"])
    b2ln = tile128(inputs["ln2_b"])
    bpb = tile128(inputs["b_proj"])
    b2b = tile128(inputs["b2"])
    b1c = np.ascontiguousarray(
        np.asarray(inputs["b1"], dtype=f).reshape(KH, 128).T)
    tril = np.tril(np.ones((128, 128), dtype=np.float32))
    mask0 = np.concatenate([tril, tril], axis=1)
    half = np.concatenate([np.ones((128, 128), dtype=np.float32), tril],
                          axis=1)
    mask1 = np.concatenate([half, half], axis=1)
    identb = np.eye(128, dtype=f)
    consts = np.ascontiguousarray(
        np.concatenate([mask0, mask1, identb], axis=1).astype(bf))

    flags = (
        bool(not np.all(np.asarray(inputs["ln1_g"]) == 1.0)),
        bool(np.any(np.asarray(inputs["ln1_b"]))),
        bool(not np.all(np.asarray(inputs["ln2_g"]) == 1.0)),
        bool(np.any(np.asarray(inputs["ln2_b"]))),
        bool(np.any(np.asarray(inputs["b_proj"]))),
        bool(np.any(np.asarray(inputs["b1"]))),
        bool(np.any(np.asarray(inputs["b2"]))),
    )
    shared = dict(
        wqkvp=wqkvp, w1=w1, w2=w2,
        g1=g1, b1ln=b1ln, g2=g2, b2ln=b2ln, bpb=bpb, b2b=b2b, b1c=b1c,
        consts=consts,
    )
    in_maps = []
    for i in range(NCORES):
        m = dict(shared)
        m["x"] = np.ascontiguousarray(x[i * BL : (i + 1) * BL])
        in_maps.append(m)
    return in_maps, flags


_NC_CACHE = {}


def _get_program(flags):
    key = (flags, _STAGE)
    if key not in _NC_CACHE:
        _NC_CACHE[key] = build_program(*flags)
    return _NC_CACHE[key]


def run(inputs, **spmd_kwargs):
    from concourse.bass_utils import run_bass_kernel_spmd

    in_maps, flags = _host_prep(inputs)
    nc = _get_program(flags)
    bkr = run_bass_kernel_spmd(nc, in_maps, list(range(NCORES)), **spmd_kwargs)
    outs = [bkr.results[i]["out"] for i in range(NCORES)]
    return np.concatenate(outs, axis=0).astype(np.float32), bkr


def kernel(**inputs):
    full, _ = run(inputs)
    return full



# revision 14
# speedup vs baseline: 1.6392x; 1.6392x over previous
"""Trainium2 Bass kernel for a dense transformer block (B=128, T=256, C=384, H=6).

Sharding: data-parallel over batch across 8 NeuronCores (16 batches/core),
identical SPMD program per core, no collectives.

Design (v3):
  - per-core schedule: batches in pairs (free dim 512 in the big matmuls),
    software-pipelined: pair p's attention exp chain (ACT/DVE) drains behind
    pair p-1's MLP (PE); input DMAs prefetched two pairs ahead so they are
    not stuck behind output DMAs in the sync queue.
  - exp on the scalar (ACT) engine (exact); causal mask + rowsum as one
    all-bf16 DVE scalar_tensor_tensor (2x rate) with accum_out.
  - 1/rowsum folded into the attention-weight transpose: regular all-bf16
    matmul with diag(1/rowsum) as the moving operand (1 cyc/row).
  - all transposes are regular matmuls against a bf16 identity (1 cyc/row).
  - relu and LN-apply on ACT (Relu / Identity with per-row scale+bias);
    LN stats + rstd (ACT Sqrt + DVE reciprocal) on DVE.
  - everything bf16 except the residual stream (x, y, out f32) and PSUM.
  - scores matmuls (K=64) must go to separate PSUM banks: two back-to-back
    K=64 matmuls into one bank crash the device (sub-array drain collision).
  - MLP down-projection i-outer so PSUM needs 1 rotating bank, not 4 held.
"""

import numpy as np

import concourse.bass as bass
import concourse.mybir as mybir
from concourse import bacc
from concourse.tile import TileContext
from contextlib import ExitStack

B, T, C = 128, 256, 384
H, D = 6, 64
FF = 4 * C
NCORES = 8
BL = B // NCORES  # 16
NPAIR = BL // 2  # 8
KC = C // 128  # 3
KH = FF // 128  # 12
EPS = 1e-5
F32 = mybir.dt.float32
F32R = mybir.dt.float32r
BF16 = mybir.dt.bfloat16
I32 = mybir.dt.int32
F8 = mybir.dt.float8e4
ALU = mybir.AluOpType
ACTF = mybir.ActivationFunctionType

EXP_S = float(2**23 / np.log(2.0))
EXP_B = float(127 * 2**23)
MASKB = 4.0e8  # masked scores -> it ~ 4e8 -> bitcast float ~1e-21 (safe to |s|<33)
SQRT_MAGIC = 0x1FBD1DF5
_STAGE = 99  # debug: truncate program after stage N (99 = full)


def build_program(use_g1, use_b1ln, use_g2, use_b2ln, use_bp, use_b1, use_b2):
    nc = bacc.Bacc(None)
    x = nc.declare_dram_parameter("x", [BL, T, C], F32, isOutput=False)
    # packed weights: [C, wq|wk|wv|wp] so one DMA trigger loads all four
    wqkvp = nc.declare_dram_parameter("wqkvp", [C, 4 * C], BF16, isOutput=False)
    w1 = nc.declare_dram_parameter("w1", [C, FF], BF16, isOutput=False)
    w2 = nc.declare_dram_parameter("w2", [FF, C], BF16, isOutput=False)
    g1 = nc.declare_dram_parameter("g1", [128, C], F32, isOutput=False)
    b1ln = nc.declare_dram_parameter("b1ln", [128, C], F32, isOutput=False)
    g2 = nc.declare_dram_parameter("g2", [128, C], F32, isOutput=False)
    b2ln = nc.declare_dram_parameter("b2ln", [128, C], F32, isOutput=False)
    bpb = nc.declare_dram_parameter("bpb", [128, C], F32, isOutput=False)
    b2b = nc.declare_dram_parameter("b2b", [128, C], F32, isOutput=False)
    b1c = nc.declare_dram_parameter("b1c", [128, KH], F32, isOutput=False)
    # packed constants: mask0 [0:256] | mask1 [256:768] | identb [768:896]
    consts = nc.declare_dram_parameter("consts", [128, 896], BF16, isOutput=False)
    out = nc.declare_dram_parameter("out", [BL, T, C], F32, isOutput=True)

    with TileContext(nc) as tc, ExitStack() as ctx:
        wts = ctx.enter_context(tc.tile_pool(name="wts", bufs=1))
        sb = ctx.enter_context(tc.tile_pool(name="sb", bufs=1))
        st = ctx.enter_context(tc.tile_pool(name="st", bufs=4))
        tr = ctx.enter_context(tc.tile_pool(name="tr", bufs=4))
        ps = ctx.enter_context(tc.tile_pool(name="ps", bufs=7, space="PSUM"))
        psy = ctx.enter_context(tc.tile_pool(name="psy", bufs=1, space="PSUM"))

        def load_one(dram, shape, tag, dt=F32):
            t_ = wts.tile(shape, dt, name=tag, tag=tag)
            nc.sync.dma_start(out=t_, in_=dram[:, :])
            return t_

        # ---- batched input DMA plan (one trigger each; sync-engine issue
        # cost is ~600ns/trigger so fewer, bigger triggers win) ----
        consts_sb = load_one(consts, [128, 896], "consts", dt=BF16)
        mask0_sb = consts_sb[:, 0:256]
        mask1_sb = consts_sb[:, 256:768]
        id_bf = consts_sb[:, 768:896]

        xp_tiles = [None] * NPAIR

        def prefetch(p):
            xp = sb.tile([128, 4 * C], F32, name="xp", tag="xp", bufs=4)
            nc.sync.dma_start(
                out=xp.rearrange("p (b t c) -> p b t c", b=2, t=2),
                in_=x[2 * p : 2 * p + 2, :, :].rearrange(
                    "b (t q) c -> q b t c", q=128
                ),
            )
            xp_tiles[p] = xp
            return [xp[:, i * C : (i + 1) * C] for i in range(4)]

        xts = [None] * NPAIR
        xts[0] = prefetch(0)
        xts[1] = prefetch(1)

        wqkvp_sb = wts.tile([128, KC * 4 * C], BF16, name="wqkvp", tag="wqkvp")
        nc.sync.dma_start(
            out=wqkvp_sb.rearrange("p (k f) -> p k f", k=KC),
            in_=wqkvp.rearrange("(k p) f -> p k f", p=128),
        )
        w3 = wqkvp_sb.rearrange("p (k f) -> p k f", k=KC)
        wq_sb = [w3[:, k, 0:C] for k in range(KC)]
        wk_sb = [w3[:, k, C : 2 * C] for k in range(KC)]
        wv_sb = [w3[:, k, 2 * C : 3 * C] for k in range(KC)]
        wp_sb = [w3[:, k, 3 * C : 4 * C] for k in range(KC)]

        xts[2] = prefetch(2)

        w1t = wts.tile([128, KC * FF], BF16, name="w1t", tag="w1t")
        nc.sync.dma_start(
            out=w1t.rearrange("p (k f) -> p k f", k=KC),
            in_=w1.rearrange("(k p) f -> p k f", p=128),
        )
        w1_3 = w1t.rearrange("p (k f) -> p k f", k=KC)
        w1_sb = [w1_3[:, k, :] for k in range(KC)]

        xts[3] = prefetch(3)

        w2t = wts.tile([128, KH * C], BF16, name="w2t", tag="w2t")
        nc.sync.dma_start(
            out=w2t.rearrange("p (m f) -> p m f", m=KH),
            in_=w2.rearrange("(m p) f -> p m f", p=128),
        )
        w2_3 = w2t.rearrange("p (m f) -> p m f", m=KH)
        w2_sb = [w2_3[:, m, :] for m in range(KH)]

        g1_sb = load_one(g1, [128, C], "g1") if use_g1 else None
        b1ln_sb = load_one(b1ln, [128, C], "b1ln") if use_b1ln else None
        g2_sb = load_one(g2, [128, C], "g2") if use_g2 else None
        b2ln_sb = load_one(b2ln, [128, C], "b2ln") if use_b2ln else None
        bpb_sb = load_one(bpb, [128, C], "bpb") if use_bp else None
        b2b_sb = load_one(b2b, [128, C], "b2b") if use_b2 else None
        b1c_sb = load_one(b1c, [128, KH], "b1c") if use_b1 else None

        def batched_rstd(mv8):
            """[128,8] interleaved (mean,var) x4 -> rstd4 [128,4]."""
            mv_v = mv8.rearrange("p (i two) -> p i two", two=2)
            var4 = mv_v[:, :, 1]
            vpe = st.tile([128, 4], F32, name="vpe", tag="vpe")
            nc.vector.tensor_scalar(
                out=vpe, in0=var4, scalar1=EPS, scalar2=None, op0=ALU.add)
            sd4 = st.tile([128, 4], F32, name="sd4", tag="sd4")
            nc.scalar.activation(sd4, vpe, ACTF.Sqrt)
            rstd4 = st.tile([128, 4], F32, name="rstd4", tag="rstd4")
            nc.vector.reciprocal(rstd4, sd4)
            return rstd4

        def ln_stat(mv8, i, src):
            stats = st.tile([128, 6], F32, name="lst", tag="lst")
            nc.vector.bn_stats(stats, src)
            nc.vector.bn_aggr(mv8[:, 2 * i : 2 * i + 2], stats)

        def layernorm4(dsts, srcs, g_sb, b_sb, mv8=None):
            if mv8 is None:
                mv8 = st.tile([128, 8], F32, name="mv8", tag="mv8")
                for i in range(4):
                    ln_stat(mv8, i, srcs[i])
            rstd4 = batched_rstd(mv8)
            mv_v2 = mv8.rearrange("p (i two) -> p i two", two=2)
            nmr4 = st.tile([128, 4], F32, name="nmr4", tag="nmr4")
            nc.vector.scalar_tensor_tensor(
                out=nmr4, in0=mv_v2[:, :, 0], scalar=-1.0, in1=rstd4,
                op0=ALU.mult, op1=ALU.mult,
            )
            for i in range(4):
                nc.scalar.activation(
                    dsts[i], srcs[i], ACTF.Identity,
                    bias=nmr4[:, i : i + 1], scale=rstd4[:, i : i + 1],
                )
                if g_sb is not None:
                    nc.vector.tensor_mul(dsts[i], dsts[i], g_sb)
                if b_sb is not None:
                    nc.vector.tensor_add(dsts[i], dsts[i], b_sb)

        def transpose4_into(dstT, srcs):
            """4x [128,C] token-major -> dstT [128, KC*2T] C-major packed."""
            dst3 = dstT.rearrange("q (c w) -> q c w", c=KC)
            for i in range(4):
                pt = ps.tile([128, C], F32, name="pa", tag="pa")
                for c in range(KC):
                    nc.tensor.matmul(
                        pt[:, c * 128 : (c + 1) * 128],
                        srcs[i][:, c * 128 : (c + 1) * 128],
                        id_bf,
                        start=True, stop=True,
                    )
                nc.scalar.copy(
                    dst3[:, :, i * 128 : (i + 1) * 128],
                    pt.rearrange("q (c w) -> q c w", c=KC),
                )


        def phase1a(p, xt):
            bs = [2 * p, 2 * p, 2 * p + 1, 2 * p + 1]
            tch = [0, 1, 0, 1]
            hT = sb.tile(
                [128, KC * 2 * T], BF16, name="hT", tag="hT", bufs=3
            )
            ht_ = [
                sb.tile([128, C], BF16, name=f"h{i}", tag=f"h{i}")
                for i in range(4)
            ]
            layernorm4(ht_, xt, g1_sb, b1ln_sb)
            transpose4_into(hT, ht_)

            # ---- stage 2: q^T (f32r), k^T (bf16) C-major; v token-major ----
            qT = [
                sb.tile([128, 2 * T], BF16, name=f"qT{m}", tag=f"qT{m}", bufs=3)
                for m in range(KC)
            ]
            kT = [
                sb.tile([128, 2 * T], BF16, name=f"kT{m}", tag=f"kT{m}", bufs=3)
                for m in range(KC)
            ]
            for m in range(KC):
                pq = ps.tile([128, 2 * T], F32, name="pa", tag="pa")
                for k in range(KC):
                    nc.tensor.matmul(
                        pq, wq_sb[k][:, m * 128 : (m + 1) * 128],
                        hT[:, k * 2 * T : (k + 1) * 2 * T],
                        start=(k == 0), stop=(k == KC - 1),
                    )
                nc.scalar.copy(qT[m], pq)
                pk = ps.tile([128, 2 * T], F32, name="pa", tag="pa")
                for k in range(KC):
                    nc.tensor.matmul(
                        pk, wk_sb[k][:, m * 128 : (m + 1) * 128],
                        hT[:, k * 2 * T : (k + 1) * 2 * T],
                        start=(k == 0), stop=(k == KC - 1),
                    )
                nc.scalar.copy(kT[m], pk)
            vt = [
                sb.tile([128, C], BF16, name=f"v{i}", tag=f"v{i}", bufs=3)
                for i in range(4)
            ]
            for i in range(4):
                pv = ps.tile([128, C], F32, name="pa", tag="pa")
                for k in range(KC):
                    nc.tensor.matmul(
                        pv, hT[:, k * 2 * T + i * 128 : k * 2 * T + (i + 1) * 128],
                        wv_sb[k],
                        start=(k == 0), stop=(k == KC - 1),
                    )
                nc.vector.tensor_copy(vt[i], pv)

            # ---- stage 3: attention ----
            # Two passes: pass A issues all score matmuls + exp chains so the
            # PE streams ahead while DVE/gpsimd chew; pass B does the
            # normalize-transposes and weight application.
            yb0_g, yb1_g, dg_g = [], [], []
            for g in range(2 * KC):
                ib, ch = g // KC, g % KC
                tb = ib * T  # token base of batch ib in 2T-packed tiles
                # NOTE: K=64 matmuls issued back-to-back into the SAME PSUM
                # bank crash the device (concurrent sub-array drains
                # collide); each head gets its own bank.
                # scores tc0: queries 0..127, keys 0..127
                pS0 = [ps.tile([128, 128], F32, name="pa", tag="pa")
                       for _ in range(2)]
                for par in range(2):
                    o = par * 64
                    nc.tensor.matmul(
                        pS0[par],
                        qT[ch][o : o + 64, tb : tb + 128],
                        kT[ch][o : o + 64, tb : tb + 128],
                        start=True, stop=True,
                    )
                ye0 = tr.tile([128, 2 * 128], BF16, name=f"ye0_{g}",
                              tag=f"ye0_{g}", bufs=1)
                for par in range(2):
                    nc.scalar.activation(
                        ye0[:, par * 128 : (par + 1) * 128], pS0[par],
                        ACTF.Exp,
                    )
                # scores tc1: queries 128..255, keys 0..255
                pS1 = [ps.tile([128, T], F32, name="pa", tag="pa")
                       for _ in range(2)]
                for par in range(2):
                    o = par * 64
                    nc.tensor.matmul(
                        pS1[par],
                        qT[ch][o : o + 64, tb + 128 : tb + 2 * 128],
                        kT[ch][o : o + 64, tb : tb + T],
                        start=True, stop=True,
                    )
                ye1 = tr.tile([128, 2 * T], BF16, name=f"ye1_{g}",
                              tag=f"ye1_{g}", bufs=1)
                for par in range(2):
                    nc.scalar.activation(
                        ye1[:, par * T : (par + 1) * T], pS1[par],
                        ACTF.Exp,
                    )
                # causal mask multiply + rowsum, all-bf16 (2x DVE rate)
                rs4 = st.tile([128, 4], F32, name=f"rs4_{g}", tag=f"rs4_{g}", bufs=1)
                yb0 = tr.tile([128, 2 * 128], BF16, name=f"yb0_{g}",
                              tag=f"yb0_{g}", bufs=1)
                yb1 = tr.tile([128, 2 * T], BF16, name=f"yb1_{g}",
                              tag=f"yb1_{g}", bufs=1)
                nc.vector.scalar_tensor_tensor(
                    out=yb0[:, 0:128], in0=ye0[:, 0:128], scalar=1.0,
                    in1=mask0_sb[:, 0:128], op0=ALU.mult, op1=ALU.mult,
                    accum_out=rs4[:, 0:1])
                nc.vector.scalar_tensor_tensor(
                    out=yb0[:, 128:256], in0=ye0[:, 128:256], scalar=1.0,
                    in1=mask0_sb[:, 128:256], op0=ALU.mult, op1=ALU.mult,
                    accum_out=rs4[:, 1:2])
                nc.vector.scalar_tensor_tensor(
                    out=yb1[:, 0:T], in0=ye1[:, 0:T], scalar=1.0,
                    in1=mask1_sb[:, 0:T], op0=ALU.mult, op1=ALU.mult,
                    accum_out=rs4[:, 2:3])
                nc.vector.scalar_tensor_tensor(
                    out=yb1[:, T : 2 * T], in0=ye1[:, T : 2 * T], scalar=1.0,
                    in1=mask1_sb[:, T : 2 * T], op0=ALU.mult, op1=ALU.mult,
                    accum_out=rs4[:, 3:4])
                rr4 = st.tile([128, 4], F32, name=f"rr4_{g}", tag=f"rr4_{g}", bufs=1)
                nc.vector.reciprocal(rr4, rs4)
                # diag(1/rowsum) in bf16 on the (otherwise idle) gpsimd
                dg = [
                    tr.tile([128, 128], BF16, name=f"dg{j}_{g}",
                            tag=f"dg{j}_{g}", bufs=1)
                    for j in range(4)
                ]
                for j in range(4):
                    nc.vector.tensor_scalar(
                        out=dg[j], in0=id_bf, scalar1=rr4[:, j : j + 1],
                        scalar2=None, op0=ALU.mult,
                    )
                yb0_g.append(yb0)
                yb1_g.append(yb1)
                dg_g.append(dg)
            return dict(bs=bs, tch=tch, xt=xt, vt=vt,
                        yb0_g=yb0_g, yb1_g=yb1_g, dg_g=dg_g)

        def phase1b(p, s):
            bs, tch, xt, vt = s["bs"], s["tch"], s["xt"], s["vt"]
            yb0_g, yb1_g, dg_g = s["yb0_g"], s["yb1_g"], s["dg_g"]
            acT = [
                sb.tile([128, 2 * T], BF16, name=f"acT{c}", tag=f"acT{c}",
                        bufs=3)
                for c in range(KC)
            ]
            for g in range(2 * KC):
                ib, ch = g // KC, g % KC
                yb0, yb1, dg = yb0_g[g], yb1_g[g], dg_g[g]
                # transpose attention weights with folded normalization:
                # regular all-bf16 matmul out = yb_slice^T @ diag(rr)
                pA = ps.tile([128, 2 * T], F32, name="pa", tag="pa")
                pB = ps.tile([128, T], F32, name="pa", tag="pa")
                # layout A: [tc0-p0 | tc1-p0-k0 | tc0-p1 | tc1-p1-k0]
                nc.tensor.matmul(pA[:, 0:128], yb0[:, 0:128], dg[0],
                                 start=True, stop=True)
                nc.tensor.matmul(pA[:, 128:256], yb1[:, 0:128], dg[2],
                                 start=True, stop=True)
                nc.tensor.matmul(pA[:, 256:384], yb0[:, 128:256], dg[1],
                                 start=True, stop=True)
                nc.tensor.matmul(pA[:, 384:512], yb1[:, 256:384], dg[3],
                                 start=True, stop=True)
                # layout B: [tc1-p0-k1 | tc1-p1-k1]
                nc.tensor.matmul(pB[:, 0:128], yb1[:, 128:256], dg[2],
                                 start=True, stop=True)
                nc.tensor.matmul(pB[:, 128:256], yb1[:, 384:512], dg[3],
                                 start=True, stop=True)
                wT0 = tr.tile([128, 2 * T], BF16, name="wT0", tag="wT0",
                              bufs=4)
                wT1 = tr.tile([128, T], BF16, name="wT1", tag="wT1",
                              bufs=4)
                nc.scalar.copy(wT0, pA)
                nc.scalar.copy(wT1, pB)
                # apply: pC[par*64:(par+1)*64, q] = sum_k v[k, d] wT[k, q]
                pC = ps.tile([128, T], F32, name="pa", tag="pa")
                for par in range(2):
                    hh = 2 * ch + par
                    o = par * 64
                    nc.tensor.matmul(
                        pC[o : o + 64, 0:T],
                        vt[ib * 2][:, hh * 64 : (hh + 1) * 64],
                        wT0[:, par * T : (par + 1) * T],
                        start=True, stop=False,
                    )
                    nc.tensor.matmul(
                        pC[o : o + 64, 128:T],
                        vt[ib * 2 + 1][:, hh * 64 : (hh + 1) * 64],
                        wT1[:, par * 128 : (par + 1) * 128],
                        start=False, stop=True,
                    )
                nc.vector.tensor_copy(acT[ch][:, ib * T : (ib + 1) * T], pC)

            # ---- stage 4: proj + residual -> y ----
            yt = [
                sb.tile([128, C], F32, name=f"y{i}", tag=f"y{i}", bufs=2)
                for i in range(4)
            ]
            mv8b = st.tile([128, 8], F32, name="mv8", tag="mv8")
            for i in range(4):
                pP = ps.tile([128, C], F32, name="pa", tag="pa")
                for k in range(KC):
                    nc.tensor.matmul(
                        pP, acT[k][:, i * 128 : (i + 1) * 128], wp_sb[k],
                        start=(k == 0), stop=(k == KC - 1),
                    )
                nc.vector.tensor_add(yt[i], pP, xt[i])
                if bpb_sb is not None:
                    nc.vector.tensor_add(yt[i], yt[i], bpb_sb)
                ln_stat(mv8b, i, yt[i])

            h2_ = [
                sb.tile([128, C], BF16, name=f"h2{i}", tag=f"h2{i}", bufs=2)
                for i in range(4)
            ]
            layernorm4(h2_, yt, g2_sb, b2ln_sb, mv8=mv8b)
            return dict(bs=bs, tch=tch, yt=yt, h2_=h2_)

        def phase2(p, s):
            bs, tch, yt, h2_ = s["bs"], s["tch"], s["yt"], s["h2_"]
            h2T = sb.tile(
                [128, KC * 2 * T], BF16, name="h2T", tag="h2T", bufs=2
            )
            transpose4_into(h2T, h2_)
            # ---- stage 6: MLP up + relu ----
            m1r = sb.tile([128, KH * 2 * T], BF16, name="m1r", tag="m1r")
            m1r3 = m1r.rearrange("p (m n) -> p m n", m=KH)
            for m in range(KH):
                pM = ps.tile([128, 2 * T], F32, name="pa", tag="pa")
                for k in range(KC):
                    nc.tensor.matmul(
                        pM, w1_sb[k][:, m * 128 : (m + 1) * 128],
                        h2T[:, k * 2 * T : (k + 1) * 2 * T],
                        start=(k == 0), stop=(k == KC - 1),
                    )
                if m % 2 == 0:
                    nc.scalar.activation(
                        m1r3[:, m, :], pM, ACTF.Relu,
                        bias=(b1c_sb[:, m : m + 1] if use_b1 else 0.0),
                    )
                else:
                    nc.vector.tensor_scalar(
                        out=m1r3[:, m, :], in0=pM,
                        scalar1=(b1c_sb[:, m : m + 1] if use_b1 else 0.0),
                        scalar2=0.0, op0=ALU.add, op1=ALU.max,
                    )

            # ---- stage 7: MLP down (i-outer) + residual + store ----
            otp = sb.tile([128, 4 * C], F32, name="otp", tag="otp", bufs=2)
            for i in range(4):
                # padded to 512 so each psY slot is bank-aligned (mm out
                # must not cross a 2KB PSUM bank)
                pY = psy.tile([128, 512], F32, name="psY", tag="psY")
                for m in range(KH):
                    nc.tensor.matmul(
                        pY[:, 0:C],
                        m1r3[:, m, i * 128 : (i + 1) * 128], w2_sb[m],
                        start=(m == 0), stop=(m == KH - 1),
                    )
                ot = otp[:, i * C : (i + 1) * C]
                nc.vector.tensor_add(ot, pY[:, 0:C], yt[i])
                if b2b_sb is not None:
                    nc.vector.tensor_add(ot, ot, b2b_sb)
            nc.sync.dma_start(
                out=out[2 * p : 2 * p + 2, :, :].rearrange(
                    "b (t q) c -> q b t c", q=128
                ),
                in_=otp.rearrange("p (b t c) -> p b t c", b=2, t=2),
            )

        # Staggered schedule: pair p-1's MLP (pure PE streaming) sits
        # BETWEEN 1a(p) (scores issued) and 1b(p) (needs exp/mask output),
        # so the PE chews MLP while pair p's exp chains drain on ACT/DVE.
        sa = [None] * NPAIR
        sb_ = [None] * NPAIR
        sa[0] = phase1a(0, xts[0])
        sb_[0] = phase1b(0, sa[0])
        for p in range(1, NPAIR):
            if p + 3 < NPAIR:
                xts[p + 3] = prefetch(p + 3)
            sa[p] = phase1a(p, xts[p])
            phase2(p - 1, sb_[p - 1])
            sb_[p] = phase1b(p, sa[p])
        phase2(NPAIR - 1, sb_[NPAIR - 1])

    nc.compile()
    return nc


def _host_prep(inputs):
    f = np.float32
    x = np.ascontiguousarray(inputs["x"], dtype=f)
    import ml_dtypes as _md

    _bf = _md.bfloat16
    wq_full = np.ascontiguousarray(
        (np.asarray(inputs["wq"], dtype=f).transpose(1, 0, 2).reshape(C, C)
         * (C ** -0.5)).astype(_bf)
    )
    wk_full = np.ascontiguousarray(
        np.asarray(inputs["wk"], dtype=f).transpose(1, 0, 2)
        .reshape(C, C).astype(_bf)
    )
    wv_full = np.ascontiguousarray(
        np.asarray(inputs["wv"], dtype=f).transpose(1, 0, 2)
        .reshape(C, C).astype(_bf)
    )
    import ml_dtypes

    bf = ml_dtypes.bfloat16
    wp = np.ascontiguousarray(np.asarray(inputs["w_proj"], dtype=f).astype(bf))
    wqkvp = np.ascontiguousarray(
        np.concatenate([wq_full, wk_full, wv_full, wp], axis=1))
    w1 = np.ascontiguousarray(np.asarray(inputs["w1"], dtype=f).astype(bf))
    w2 = np.ascontiguousarray(np.asarray(inputs["w2"], dtype=f).astype(bf))
    tile128 = lambda v: np.ascontiguousarray(
        np.broadcast_to(np.asarray(v, dtype=f), (128, C))
    )
    g1 = tile128(inputs["ln1_g"])
    b1ln = tile128(inputs["ln1_b"])
    g2 = tile128(inputs["ln2_g"])
    b2ln = tile128(inputs["ln2_b"])
    bpb = tile128(inputs["b_proj"])
    b2b = tile128(inputs["b2"])
    b1c = np.ascontiguousarray(
        np.asarray(inputs["b1"], dtype=f).reshape(KH, 128).T)
    tril = np.tril(np.ones((128, 128), dtype=np.float32))
    mask0 = np.concatenate([tril, tril], axis=1)
    half = np.concatenate([np.ones((128, 128), dtype=np.float32), tril],
                          axis=1)
    mask1 = np.concatenate([half, half], axis=1)
    identb = np.eye(128, dtype=f)
    consts = np.ascontiguousarray(
        np.concatenate([mask0, mask1, identb], axis=1).astype(bf))

    flags = (
        bool(not np.all(np.asarray(inputs["ln1_g"]) == 1.0)),
        bool(np.any(np.asarray(inputs["ln1_b"]))),
        bool(not np.all(np.asarray(inputs["ln2_g"]) == 1.0)),
        bool(np.any(np.asarray(inputs["ln2_b"]))),
        bool(np.any(np.asarray(inputs["b_proj"]))),
        bool(np.any(np.asarray(inputs["b1"]))),
        bool(np.any(np.asarray(inputs["b2"]))),
    )
    shared = dict(
        wqkvp=wqkvp, w1=w1, w2=w2,
        g1=g1, b1ln=b1ln, g2=g2, b2ln=b2ln, bpb=bpb, b2b=b2b, b1c=b1c,
        consts=consts,
    )
    in_maps = []
    for i in range(NCORES):
        m = dict(shared)
        m["x"] = np.ascontiguousarray(x[i * BL : (i + 1) * BL])
        in_maps.append(m)
    return in_maps, flags


_NC_CACHE = {}


def _get_program(flags):
    key = (flags, _STAGE)
    if key not in _NC_CACHE:
        _NC_CACHE[key] = build_program(*flags)
    return _NC_CACHE[key]


def run(inputs, **spmd_kwargs):
    from concourse.bass_utils import run_bass_kernel_spmd

    in_maps, flags = _host_prep(inputs)
    nc = _get_program(flags)
    bkr = run_bass_kernel_spmd(nc, in_maps, list(range(NCORES)), **spmd_kwargs)
    outs = [bkr.results[i]["out"] for i in range(NCORES)]
    return np.concatenate(outs, axis=0).astype(np.float32), bkr


def kernel(**inputs):
    full, _ = run(inputs)
    return full

